# revision 4
# baseline (speedup 1.0000x reference)
"""Int8Linear (rowwise-quant activation x int8 weight GEMM) on 8 TRN2 cores.

Strategy: data-parallel over tokens (M). Each core gets M/8 = 1024 rows of x,
quantizes them rowwise on-device, transposes the quantized activations into
K-major layout via PE transposes (A^T stays SBUF-resident, 8 MiB bf16), then
streams the host-pre-transposed bf16 weight matrix W^T [K, N] through the
tensor engine: psum[m128, n512] += A^T_tile.T @ W^T_tile over k.
Epilogue fuses (psum * scale_a[m]) * wscale[n] + bias[n] into 2 DVE ops using
host-pre-broadcast [128, N] wscale/bias planes.

bf16 is exact for integers in [-127, 127], and fp32 PSUM accumulation of
integer products is exact below 2^24, so the int8 GEMM is bit-exact.
"""

import os
import numpy as np
import ml_dtypes

import concourse.bacc as bacc
import concourse.mybir as mybir
from concourse import tile
from concourse.bass_utils import run_bass_kernel_spmd
from concourse.masks import make_identity

P = 128
QMAX = 127.0
EPS = 1e-8
MAGIC = 12582912.0  # 1.5 * 2**23: (x + MAGIC) - MAGIC == round-half-even(x)

M, K, N = 8192, 4096, 16384
NCORES = 8
MS = M // NCORES  # 1024 rows per core

FP32 = mybir.dt.float32
BF16 = mybir.dt.bfloat16


def build_nc(ms=MS, k=K, n=N, wt_bufs=12, acc_bufs=4, n_tile=512):
    """Emit the per-core SPMD kernel. All cores run the same program."""
    mt_cnt = ms // P          # m-subtiles per core
    kt_cnt = k // P           # 128-row k-subtiles
    ko_cnt = max(1, k // 512) # k-outer DMA blocks
    ks_cnt = kt_cnt // ko_cnt # k-subtiles per DMA block (<= 4)
    nb_cnt = n // n_tile      # n blocks

    nc = bacc.Bacc(
        "TRN2",
        target_bir_lowering=False,
        debug=False,
        enable_asserts=False,
        num_devices=NCORES,
    )
    x_d = nc.dram_tensor("x", [ms, k], FP32, kind="ExternalInput")
    wt_d = nc.dram_tensor("wt", [k, n], BF16, kind="ExternalInput")
    wsb_d = nc.dram_tensor("wsb", [P, n], FP32, kind="ExternalInput")
    bsb_d = nc.dram_tensor("bsb", [P, n], FP32, kind="ExternalInput")
    out_d = nc.dram_tensor("out", [ms, n], FP32, kind="ExternalOutput")

    with tile.TileContext(nc) as tc:
        with (
            tc.tile_pool(name="const", bufs=1) as const,
            tc.tile_pool(name="xp", bufs=2) as xp,
            tc.tile_pool(name="abp", bufs=2) as abp,
            tc.tile_pool(name="sc", bufs=2) as sc,
            tc.tile_pool(name="tp", bufs=2, space="PSUM") as tp,
            tc.tile_pool(name="wtp", bufs=wt_bufs) as wtp,
            tc.tile_pool(name="wbp", bufs=2) as wbp,
            tc.tile_pool(name="acc", bufs=acc_bufs, space="PSUM") as accp,
            tc.tile_pool(name="ep", bufs=4) as ep,
        ):
            ident = const.tile([P, P], BF16)
            make_identity(nc, ident)
            sa_all = const.tile([P, mt_cnt], FP32)          # scale_a, col per m-tile
            at = const.tile([P, kt_cnt, ms], BF16)          # A^T resident

            # ---- Phase A: rowwise quantize + transpose into at ----
            for mt in range(mt_cnt):
                xt = xp.tile([P, k], FP32, tag="x")
                nc.sync.dma_start(out=xt, in_=x_d[mt * P:(mt + 1) * P, :])
                amax = sc.tile([P, 1], FP32, tag="amax")
                nc.vector.tensor_reduce(
                    out=amax, in_=xt, axis=mybir.AxisListType.X,
                    op=mybir.AluOpType.max, apply_absolute_value=True,
                )
                # sa = max(amax * (1/127), eps)
                sa_col = sa_all[:, mt:mt + 1]
                nc.vector.tensor_scalar(
                    out=sa_col, in0=amax, scalar1=1.0 / QMAX, scalar2=EPS,
                    op0=mybir.AluOpType.mult, op1=mybir.AluOpType.max,
                )
                # rsa = 1/sa with one Newton step: rsa*(2 - sa*rsa)
                rsa = sc.tile([P, 1], FP32, tag="rsa")
                nc.vector.reciprocal(out=rsa, in_=sa_col)
                rerr = sc.tile([P, 1], FP32, tag="rerr")
                nc.vector.scalar_tensor_tensor(
                    out=rerr, in0=sa_col, scalar=-1.0, in1=rsa,
                    op0=mybir.AluOpType.mult, op1=mybir.AluOpType.mult,
                )
                nc.vector.tensor_scalar(
                    out=rerr, in0=rerr, scalar1=2.0, scalar2=None,
                    op0=mybir.AluOpType.add,
                )
                nc.vector.tensor_tensor(
                    out=rsa, in0=rsa, in1=rerr, op=mybir.AluOpType.mult)
                # xt = x * rsa + MAGIC  (in place)
                nc.vector.tensor_scalar(
                    out=xt, in0=xt, scalar1=rsa, scalar2=MAGIC,
                    op0=mybir.AluOpType.mult, op1=mybir.AluOpType.add,
                )
                # ab = bf16(xt - MAGIC) -- exact integers in [-127, 127]
                ab = abp.tile([P, k], BF16, tag="ab")
                nc.vector.tensor_scalar(
                    out=ab, in0=xt, scalar1=MAGIC, scalar2=None,
                    op0=mybir.AluOpType.subtract,
                )
                for kk in range(kt_cnt):
                    pt = tp.tile([P, P], BF16, tag="tp")
                    nc.tensor.transpose(pt, ab[:, kk * P:(kk + 1) * P], ident)
                    nc.vector.tensor_copy(
                        out=at[:, kk, mt * P:(mt + 1) * P], in_=pt)

            # ---- Phase B: GEMM + epilogue ----
            for nb in range(nb_cnt):
                n0 = nb * n_tile
                wsb = wbp.tile([P, n_tile], FP32, tag="wsb")
                nc.sync.dma_start(out=wsb, in_=wsb_d[:, n0:n0 + n_tile])
                bsb = wbp.tile([P, n_tile], FP32, tag="bsb")
                nc.sync.dma_start(out=bsb, in_=bsb_d[:, n0:n0 + n_tile])
                wts = []
                for ko in range(ko_cnt):
                    wt_t = wtp.tile([P, ks_cnt, n_tile], BF16, tag="wt")
                    nc.sync.dma_start(
                        out=wt_t,
                        in_=wt_d[ko * ks_cnt * P:(ko + 1) * ks_cnt * P,
                                 n0:n0 + n_tile].rearrange(
                                     "(s p) n -> p s n", p=P),
                    )
                    wts.append(wt_t)
                for mb in range(mt_cnt):
                    ps = accp.tile([P, n_tile], FP32, tag="acc")
                    for ko in range(ko_cnt):
                        for ks in range(ks_cnt):
                            ki = ko * ks_cnt + ks
                            nc.tensor.matmul(
                                ps,
                                lhsT=at[:, ki, mb * P:(mb + 1) * P],
                                rhs=wts[ko][:, ks, :],
                                start=(ki == 0), stop=(ki == kt_cnt - 1),
                            )
                    # out = (psum * sa) * wscale + bias
                    ot = ep.tile([P, n_tile], FP32, tag="ot")
                    nc.vector.scalar_tensor_tensor(
                        out=ot, in0=ps, scalar=sa_all[:, mb:mb + 1], in1=wsb,
                        op0=mybir.AluOpType.mult, op1=mybir.AluOpType.mult,
                    )
                    nc.vector.tensor_add(ot, ot, bsb)
                    nc.sync.dma_start(
                        out=out_d[mb * P:(mb + 1) * P, n0:n0 + n_tile], in_=ot)
    nc.finalize()
    return nc


def host_prep(x, weight_int8, weight_scales, bias):
    """Layout-only host prep: shard x, pre-transpose/cast weights, broadcast
    the per-channel vectors to [128, N] planes."""
    x = np.ascontiguousarray(np.asarray(x, dtype=np.float32))
    w = np.asarray(weight_int8)
    if w.dtype != np.int8:
        w = w.astype(np.int8)
    wt = np.ascontiguousarray(w.T).astype(ml_dtypes.bfloat16)  # [K, N]
    ws = np.asarray(weight_scales, dtype=np.float32).reshape(1, -1)
    bs = np.asarray(bias, dtype=np.float32).reshape(1, -1)
    wsb = np.ascontiguousarray(np.broadcast_to(ws, (P, ws.shape[1])))
    bsb = np.ascontiguousarray(np.broadcast_to(bs, (P, bs.shape[1])))
    in_maps = []
    for c in range(NCORES):
        in_maps.append({
            "x": x[c * MS:(c + 1) * MS],
            "wt": wt,
            "wsb": wsb,
            "bsb": bsb,
        })
    return in_maps


_CACHE = {}
LAST_EXEC_NS = None
LAST_PROFILE = None


def kernel(x, weight_int8, weight_scales, bias):
    global LAST_EXEC_NS, LAST_PROFILE
    if "nc" not in _CACHE:
        _CACHE["nc"] = build_nc()
    nc = _CACHE["nc"]
    in_maps = host_prep(x, weight_int8, weight_scales, bias)
    trace = bool(int(os.environ.get("K_TRACE", "0")))
    res = run_bass_kernel_spmd(nc, in_maps, list(range(NCORES)), trace=trace)
    LAST_EXEC_NS = res.exec_time_ns
    LAST_PROFILE = getattr(res, "profile_json", None)
    out = np.concatenate([r["out"] for r in res.results], axis=0)
    return out
